# revision 10
# baseline (speedup 1.0000x reference)
"""Trainium2 Bass kernel for a GPT-style transformer block (v6).

v6/v7 changes vs v3 (each step verified on HW, 727us -> 627us):
  - the h^T AllGather payload is quantized to fp8 e4m3. Halves the
    16MB gather to 8MB, cutting the CC-stream serial time that gates
    qkv (-54us). Measured rel err with this alone IMPROVED to 1.67e-2.
  - qkv runs fp8 DoubleRow end-to-end (-34us PE): the gathered h is
    already fp8; wq/wk/wv are packed host-side as fp8 DR pairs scaled
    by SFQ=2^k (pow2scale to ~192 — w_attn ~N(0,0.02) sits in e4m3's
    subnormal range unscaled). The unscale rides the existing eviction
    activations for free (k/v: scale=1/SFQ; q: the qsc 0/1 mask rows
    hold 1/SFQ). Total measured rel err 1.83e-2 < 2e-2 gate.
  - the diag-tile causal mask-init PE matmuls are GONE (-20us PE):
    diag QK runs start=True on the live columns and the causal
    boundary is applied post-exp as a 0/1 triangle multiply on DVE
    (tri[p,jj] = jj>=p, one [128,128] const), with a memset for the
    one uninitialized gap band per unit (k2=1's [128*s0, 128*s0+128))
    so no exp-of-garbage value ever reaches the AV matmul.
  - the joint v-natural tiles (vnb/vn8) are built once per batch at the
    tail of ph2 (was: per (h, b) — 2x transposes/copies and a 1-5us PE
    stall at each (h, b) start).
  - psQK bufs 2->3 (deeper QK->exp->AV pipelining); wproj load deferred
    to ph3 so it stops delaying the ph1 x/h DMAs.
  - (tried and reverted: a t=0 warm-up collective — the CC stream only
    becomes usable ~60-90us into the NEFF regardless, so the warm-up
    just delays AG0 by its own ~13us of serial CC time. Also reverted:
    fp8 y^T AllToAll — with bf16 qkv it measured 1.65e-2, but stacked
    on the fp8-DR qkv it lands at 2.0017e-2, OVER the 2e-2 gate. Do
    not retry. Remaining
    exposed time is CC-bound: ~59us AG0 stream-latency wait, ~36us y1
    AllToAll wait, ~25us NEFF startup, ~15us epilogue; ph5 (fc+fc2)
    runs at 96% PE occupancy, at the N+128-cycle instruction floor.)

Reference computation (B=4, T=2048, d=1024, 16 heads, dff=4096, fp32):
    h  = LN1(x);  qkv = h @ w_attn + b_attn
    y  = causal_attention(q, k, v);  x1 = x + y @ w_proj + b_proj
    h2 = LN2(x1); out = x1 + gelu(h2 @ w_fc + b_fc) @ w_fc2 + b_fc2

Sharding over 8 NeuronCores (one trn2 chip), same as v1: head-parallel
attention (core c owns heads 2c, 2c+1) with an AllGather of the LN1 output
h^T; per-head y^T AllToAll back to token-sharded form; proj/LN2/MLP
token-parallel (core c owns flattened tokens [1024c, 1024c+1024)).

v3 design points (from v1/v2 profiling + an fp8 error study):
  - fp8 e4m3 + MatmulPerfMode.DoubleRow (2x matmul throughput, validated
    207ns vs 2x222ns for K=256,N=512) is used ONLY for the fc2 GEMM:
    gelu(m) @ w_fc2 quantization noise is a Gaussian sum over K=4096
    (~1.3e-2 final rel err, measured 1.76e-2 total on HW). fp8 activations
    anywhere in the attention path fail the 2e-2 gate: causally-early
    tokens have sharp attention, so any 3.6% fp8 noise on h/q/k/v/y lands
    undamped on the residual stream (measured ~0.8e-2 per source in a
    numpy study) — so LN1/qkv/QK/proj stay bf16. Exception: the AV matmul
    runs fp8 DoubleRow for tq>=1 query tiles only — scores have std 0.41,
    so rows with >=513 causal positions are provably flat (p_max < 2%) and
    the v/exp quantization noise cancels; tq=0 keeps bf16 AV.
  - the runtime's global barrier rides on the first CC op and cannot start
    before ~21us (fixed NEFF startup latency); LN1 is fast enough that the
    first h AllGather triggers it with little extra exposure. (A separate
    warm-up collective was tried and removed: each CC op costs ~10-15us of
    serial CC-stream time, more than it saved.)
  - LN gamma/beta are folded into the following matmul weights host-side
    (on-chip layernorm is standardization only); proj bias is folded into
    the residual input host-side (x_res), removing per-tile bias work.
  - layernorm transposes run in bf16 (1 cycle/row vs 2 for fp32) with the
    PSUM->SBUF eviction on DVE; LN variance reciprocal uses
    reciprocal_approx_fast (SBUF [128,1], validated); the softmax
    normalizer stages the PSUM sums into SBUF and uses
    reciprocal_approx_fast there (~4x cheaper than nc.vector.reciprocal,
    which cost 129us total; the approx op on PSUM operands is unproven).
"""

import sys

import numpy as np
import ml_dtypes

sys.path.insert(0, "/opt/trn_rl_repo")

import concourse.bass as bass  # noqa: E402
import concourse.mybir as mybir  # noqa: E402
import concourse.tile as tile  # noqa: E402
from concourse import bacc  # noqa: E402
from concourse.bass_utils import run_bass_kernel_spmd  # noqa: E402
from concourse.masks import make_identity  # noqa: E402

B, T, D, H, HD, DFF = 4, 2048, 1024, 16, 64, 4096
EPS = 1e-5
NCORES = 8
TOK = B * T
TOWN = TOK // NCORES
P = 128
QT = TOWN // 2  # 512 tokens per AllGather half
F32 = mybir.dt.float32
BF16 = mybir.dt.bfloat16
FP8 = mybir.dt.float8e4
Act = mybir.ActivationFunctionType
Alu = mybir.AluOpType
AX = mybir.AxisListType
DR = mybir.MatmulPerfMode.DoubleRow
NEG = -1.0e9
BF = ml_dtypes.bfloat16
F8 = ml_dtypes.float8_e4m3


def build(SF2, SFQ):
    nc = bacc.Bacc("TRN2", target_bir_lowering=False, debug=False, num_devices=NCORES)

    def inp(name, shape, dt=F32):
        return nc.dram_tensor(name, shape, dt, kind="ExternalInput").ap()

    x_own = inp("x_own", [TOWN, D])          # pristine x shard (LN1 input)
    x_res = inp("x_res", [TOWN, D])          # x + b_proj (proj residual base)
    wq = inp("wq", [P, 4, 2, P], FP8)    # SFQ * w, DR pair layout
    wk = inp("wk", [P, 4, 2, P], FP8)
    wv = inp("wv", [P, 4, 2, P], FP8)
    qsc = inp("qsc", [P, 2])                 # per-head 0/1 mask
    qbi = inp("qbi", [P, 2])                 # per-head masked q bias
    bk = inp("bk", [P, 1])
    bv = inp("bv", [P, 1])
    wproj = inp("wproj", [D, D], BF16)
    wfc = inp("wfc", [D, DFF], BF16)         # ln2-folded
    bfc = inp("bfc", [P, DFF // P])
    wfc28 = inp("wfc28", [P, 16, 2, D], FP8)     # SF2 * w_fc2, pair layout
    bfc2s = inp("bfc2s", [1, D], BF16)           # SF2 * b_fc2
    out_own = nc.dram_tensor("out", [TOWN, D], F32, kind="ExternalOutput").ap()

    groups = [list(range(NCORES))]

    with tile.TileContext(nc) as tc:
        with (
            tc.tile_pool(name="const", bufs=1) as cst,
            tc.tile_pool(name="dram", bufs=1, space="DRAM") as dram,
        ):
            # ---------------- constants ----------------
            ident_bf = cst.tile([P, P], BF16)
            make_identity(nc, ident_bf)
            ones_b = cst.tile([1, P], BF16)
            nc.vector.memset(ones_b[:], 1.0)
            qsc_sb = cst.tile([P, 2], F32)
            nc.sync.dma_start(qsc_sb[:], qsc)
            qbi_sb = cst.tile([P, 2], F32)
            nc.sync.dma_start(qbi_sb[:], qbi)
            bk_sb = cst.tile([P, 1], F32)
            nc.sync.dma_start(bk_sb[:], bk)
            bv_sb = cst.tile([P, 1], F32)
            nc.sync.dma_start(bv_sb[:], bv)
            bfc_sb = cst.tile([P, DFF // P], F32)
            nc.sync.dma_start(bfc_sb[:], bfc)
            bfc2_sb = cst.tile([1, D], BF16)
            nc.sync.dma_start(bfc2_sb[:], bfc2s)
            # causal 0/1 triangle for the diag bands, applied as a DVE
            # multiply on the exp output (replaces the per-diag-unit
            # mask-init PE matmuls entirely: ~25us of PE). tri[p, jj] = 1
            # iff jj >= p (live), 0 above the diagonal.
            tri_b = cst.tile([P, P], BF16)
            tri_8 = cst.tile([P, P], FP8)
            with tc.tile_pool(name="mtmp", bufs=1) as mtmp:
                trif = mtmp.tile([P, P], F32)
                nc.vector.memset(trif[:], 1.0)
                nc.gpsimd.affine_select(
                    out=trif[:],
                    in_=trif[:],
                    pattern=[[1, P]],
                    channel_multiplier=-1,
                    base=0,
                    compare_op=Alu.is_ge,
                    fill=0.0,
                )
                nc.vector.tensor_copy(tri_b[:], trif[:])
                nc.vector.tensor_copy(tri_8[:], trif[:])

            # DRAM intermediates
            hT_dram_q = [dram.tile([D, QT], FP8, name=f"hq{i}") for i in range(2)]
            hT_full_q = [dram.tile([NCORES * D, QT], FP8, addr_space="Shared",
                                   name=f"hfq{i}") for i in range(2)]
            yT_send = [dram.tile([NCORES, HD, TOWN], BF16, name=f"ys{h}")
                       for h in range(2)]
            yT_recv = [dram.tile([NCORES, HD, TOWN], BF16, name=f"yr{h}")
                       for h in range(2)]

            # =========================================================
            # shared LN helper: standardize a [128, D] token tile and
            # write transposed blocks into dstT[:, dblk, t, :]
            # =========================================================
            def layernorm_std(pool, xt, ps_pool, dstT, t):
                ssum = pool.tile([P, 1], F32, tag="ssum")
                nc.vector.reduce_sum(ssum[:], xt[:], axis=AX.X)
                mean = pool.tile([P, 1], F32, tag="mean")
                nc.scalar.mul(mean[:], ssum[:], 1.0 / D)
                sq = pool.tile([P, D], F32, tag="sq")
                sumsq = pool.tile([P, 1], F32, tag="sumsq")
                nc.scalar.activation(sq[:], xt[:], Act.Square, accum_out=sumsq[:])
                msq = pool.tile([P, 1], F32, tag="msq")
                nc.vector.tensor_tensor(msq[:], mean[:], mean[:], op=Alu.mult)
                var = pool.tile([P, 1], F32, tag="var")
                nc.vector.tensor_scalar(var[:], sumsq[:], 1.0 / D, EPS, Alu.mult, Alu.add)
                nc.vector.tensor_tensor(var[:], var[:], msq[:], op=Alu.subtract)
                rinv = pool.tile([P, 1], F32, tag="rinv")
                nc.vector.reciprocal_approx_fast(rinv[:], var[:])
                rstd = pool.tile([P, 1], F32, tag="rstd")
                nc.scalar.sqrt(rstd[:], rinv[:])
                hh = pool.tile([P, D], BF16, tag="hh")
                nc.vector.tensor_scalar(
                    hh[:], xt[:], mean[:], rstd[:], Alu.subtract, Alu.mult
                )
                for half in range(2):
                    ptb = ps_pool.tile([P, 4, P], BF16, tag="lnt")
                    for q2 in range(4):
                        dblk = 4 * half + q2
                        nc.tensor.transpose(
                            ptb[:, q2, :], hh[:, dblk * P : (dblk + 1) * P],
                            ident_bf[:],
                        )
                    nc.vector.tensor_copy(
                        dstT[:, 4 * half : 4 * half + 4, t, :], ptb[:]
                    )

            # =========================================================
            # Phase 1: LN1 (standardize only) -> h^T bf16 -> AllGather x2
            # =========================================================
            with (
                tc.tile_pool(name="ph1", bufs=4) as ph1,
                tc.tile_pool(name="ph1T", bufs=1) as ph1T,
                tc.tile_pool(name="psA", bufs=4, space="PSUM") as psA,
                nc.named_scope("ph1_ln1"),
            ):
                hT_asm = [ph1T.tile([P, 8, 4, P], FP8, name=f"hasm{i}")
                          for i in range(2)]
                for qi in range(2):
                    for tl in range(4):
                        t = 4 * qi + tl
                        xt = ph1.tile([P, D], F32, tag="xt")
                        nc.sync.dma_start(xt[:], x_own[t * P : (t + 1) * P, :])
                        layernorm_std(ph1, xt, psA, hT_asm[qi], tl)
                    hTv = hT_dram_q[qi].rearrange("(dblk p) t -> p dblk t", p=P)
                    for dblk in range(8):
                        nc.sync.dma_start(
                            hTv[:, dblk, :], hT_asm[qi][:, dblk, :, :]
                        )
                    nc.gpsimd.collective_compute(
                        "AllGather", Alu.bypass, replica_groups=groups,
                        ins=[hT_dram_q[qi][:]], outs=[hT_full_q[qi][:]],
                    )

            # =========================================================
            # Phase 2: qkv (bf16) for this core's two heads over all
            # tokens. tile16 = rr*2 + qi; token = tile16*512 + j
            # =========================================================
            with tc.tile_pool(name="wkeep", bufs=1) as wkeep:
                wfc2_sb = wkeep.tile([P, 16, 2, D], FP8)
                wproj_sb = wkeep.tile([P, 8, D], BF16)
                attn_scope = tc.tile_pool(name="qkv", bufs=1)
                qkvp = attn_scope.__enter__()
                qTp = [qkvp.tile([P, 16, 512], BF16, name=f"qTp{h}") for h in range(2)]
                kT = qkvp.tile([P, 16, 512], BF16)
                vT = qkvp.tile([P, 16, 512], BF16)
                # joint v natural [kv, {v0, ones}|{v1, ones}] per batch,
                # built ONCE per batch at the tail of ph2 (shared by both
                # heads; rebuilding per (h, b) doubled the transposes and
                # stalled PE at every (h, b) start)
                vn8_b = [qkvp.tile([P, 16, 2 * P], FP8, name=f"vn8_{b}")
                         for b in range(B)]
                vnb_b = [qkvp.tile([P, 4, 2 * P], BF16, name=f"vnb_{b}")
                         for b in range(B)]
                with (
                    tc.tile_pool(name="wqkv", bufs=1) as wp,
                    tc.tile_pool(name="ph2", bufs=3) as ph2,
                    tc.tile_pool(name="psB", bufs=4, space="PSUM") as psB,
                    tc.tile_pool(name="psT2", bufs=2, space="PSUM") as psT2,
                    nc.named_scope("ph2_qkv"),
                ):
                    wq_sb = wp.tile([P, 4, 2, P], FP8)
                    nc.sync.dma_start(wq_sb[:], wq)
                    wk_sb = wp.tile([P, 4, 2, P], FP8)
                    nc.sync.dma_start(wk_sb[:], wk)
                    wv_sb = wp.tile([P, 4, 2, P], FP8)
                    nc.sync.dma_start(wv_sb[:], wv)
                    hfvs = [hq.rearrange("(r dblk p) t -> r p dblk t", p=P, dblk=8)
                            for hq in hT_full_q]
                    for qi in range(2):
                        for rr in range(8):
                            ht = ph2.tile([P, 8, QT], FP8, tag="ht")
                            nc.sync.dma_start(ht[:], hfvs[qi][rr])
                            t16 = rr * 2 + qi
                            for wi, w_sb in enumerate((wq_sb, wk_sb, wv_sb)):
                                ps = psB.tile([P, QT], F32, tag="qkvps")
                                for ko2 in range(4):
                                    nc.tensor.matmul(
                                        ps[:], w_sb[:, ko2, :, :],
                                        ht[:, 2 * ko2 : 2 * ko2 + 2, :],
                                        start=(ko2 == 0), stop=(ko2 == 3),
                                        perf_mode=DR,
                                        skip_group_check=True,
                                    )
                                if wi == 0:
                                    for h in range(2):
                                        nc.scalar.activation(
                                            qTp[h][:, t16, :], ps[:],
                                            Act.Identity,
                                            bias=qbi_sb[:, h : h + 1],
                                            scale=qsc_sb[:, h : h + 1],
                                        )
                                elif wi == 1:
                                    nc.scalar.activation(
                                        kT[:, t16, :], ps[:], Act.Identity,
                                        bias=bk_sb[:], scale=1.0 / SFQ,
                                    )
                                else:
                                    nc.scalar.activation(
                                        vT[:, t16, :], ps[:], Act.Identity,
                                        bias=bv_sb[:], scale=1.0 / SFQ,
                                    )
                            if qi == 1 and rr % 2 == 1:
                                b = (rr - 1) // 2
                                vnb, vn8 = vnb_b[b], vn8_b[b]
                                nc.vector.memset(vnb[:, :, HD:P], 1.0)
                                nc.vector.memset(vnb[:, :, P + HD : 2 * P], 1.0)
                                nc.vector.memset(vn8[:, :, HD:P], 1.0)
                                nc.vector.memset(vn8[:, :, P + HD : 2 * P], 1.0)
                                for kq in range(4):
                                    ptb = psT2.tile([P, 4, P], BF16, tag="vtp")
                                    for k2 in range(4):
                                        kb = kq * 4 + k2
                                        nc.tensor.transpose(
                                            ptb[:, k2, :],
                                            vT[:, b * 4 + kb // 4,
                                               (kb % 4) * P : (kb % 4 + 1) * P],
                                            ident_bf[:],
                                        )
                                    nc.vector.tensor_copy(
                                        vn8[:, kq * 4 : (kq + 1) * 4, :]
                                        .rearrange("p t (g c) -> p t g c", g=2)
                                        [:, :, :, 0:HD],
                                        ptb[:].rearrange("p t (g c) -> p t g c", g=2),
                                    )
                                    if kq == 0:
                                        nc.vector.tensor_copy(
                                            vnb[:]
                                            .rearrange("p t (g c) -> p t g c", g=2)
                                            [:, :, :, 0:HD],
                                            ptb[:].rearrange("p t (g c) -> p t g c", g=2),
                                        )

                # =====================================================
                # Phase 3: causal attention, 8 (batch, head) units
                # =====================================================
                with (
                    tc.tile_pool(name="ph3", bufs=6) as ph3,
                    tc.tile_pool(name="ph3s", bufs=4) as ph3s,
                    tc.tile_pool(name="psQK", bufs=3, space="PSUM") as psQK,
                    tc.tile_pool(name="psY", bufs=2, space="PSUM") as psY,
                    nc.named_scope("ph3_attn"),
                ):
                    nc.sync.dma_start(wfc2_sb[:], wfc28)
                    nc.sync.dma_start(
                        wproj_sb[:], wproj.rearrange("(ko p) n -> p ko n", p=P)
                    )
                    for h in range(2):
                        for b in range(B):
                            vnb, vn8 = vnb_b[b], vn8_b[b]
                            vn8p = vn8[:].rearrange("p (u i) c -> p u i c", i=2)
                            for tq in range(4):
                                nu = 2 * (tq + 1)
                                py = psY.tile([P, 512], F32, tag="py")
                                pend = []
                                for u in range(nu):
                                    ps = psQK.tile([P, 2, 512], F32, tag="qk")
                                    diag = u >= 2 * tq
                                    off = 128 * max(0, 2 * (u - 2 * tq))
                                    for k2 in range(2):
                                        kb = 2 * u + k2
                                        if diag:
                                            s = kb - 4 * tq
                                            nc.tensor.matmul(
                                                ps[:, k2, 128 * s :],
                                                kT[:, b * 4 + kb // 4,
                                                   (kb % 4) * P : (kb % 4 + 1) * P],
                                                qTp[h][:, b * 4 + tq, 128 * s :],
                                                start=True, stop=True,
                                            )
                                        else:
                                            nc.tensor.matmul(
                                                ps[:, k2, :],
                                                kT[:, b * 4 + kb // 4,
                                                   (kb % 4) * P : (kb % 4 + 1) * P],
                                                qTp[h][:, b * 4 + tq, :],
                                                start=True, stop=True,
                                            )
                                    exdt = BF16 if tq == 0 else FP8
                                    ex = ph3.tile([P, 2, 512], exdt,
                                                  tag="ex" if tq == 0 else "ex8")
                                    nc.scalar.activation(
                                        ex[:, :, off:], ps[:, :, off:], Act.Exp,
                                        scale=1.0 / np.sqrt(HD),
                                    )
                                    if diag:
                                        # k2=1's [128s0,128s0+128) cols are
                                        # exp of uninitialized PSUM: zero
                                        # them (memset, no NaN path), then
                                        # 0/1-triangle each diag band
                                        s0 = 2 * (u - 2 * tq)
                                        tri = tri_b if tq == 0 else tri_8
                                        nc.vector.memset(
                                            ex[:, 1, 128 * s0 : 128 * s0 + P],
                                            0.0,
                                        )
                                        for k2 in range(2):
                                            sb_ = 128 * (s0 + k2)
                                            nc.vector.tensor_tensor(
                                                ex[:, k2, sb_ : sb_ + P],
                                                ex[:, k2, sb_ : sb_ + P],
                                                tri[:],
                                                op=Alu.mult,
                                            )
                                    pend.append((u, off, ex))
                                    if len(pend) > 2:  # AV trails 2 pairs
                                        u0, o0, e0 = pend.pop(0)
                                        if tq == 0:
                                            for k2 in range(2):
                                                kb = 2 * u0 + k2
                                                nc.tensor.matmul(
                                                    py[:, o0:],
                                                    vnb[:, kb, h * P : (h + 1) * P],
                                                    e0[:, k2, o0:],
                                                    start=(kb == 0), stop=False,
                                                )
                                        else:
                                            nc.tensor.matmul(
                                                py[:, o0:],
                                                vn8p[:, u0, :, h * P : (h + 1) * P],
                                                e0[:, :, o0:],
                                                start=(u0 == 0), stop=False,
                                                perf_mode=DR,
                                                skip_group_check=True,
                                            )
                                while pend:
                                    u0, o0, e0 = pend.pop(0)
                                    if tq == 0:
                                        for k2 in range(2):
                                            kb = 2 * u0 + k2
                                            nc.tensor.matmul(
                                                py[:, o0:],
                                                vnb[:, kb, h * P : (h + 1) * P],
                                                e0[:, k2, o0:],
                                                start=(kb == 0),
                                                stop=(not pend and k2 == 1),
                                            )
                                    else:
                                        nc.tensor.matmul(
                                            py[:, o0:],
                                            vn8p[:, u0, :, h * P : (h + 1) * P],
                                            e0[:, :, o0:],
                                            start=(u0 == 0),
                                            stop=(not pend),
                                            perf_mode=DR,
                                            skip_group_check=True,
                                        )
                                # normalize: rows 64..127 hold softmax sums
                                # (stage sums in SBUF; approx reciprocal's
                                # custom bitwise ops proven on SBUF only)
                                ssb = ph3s.tile([HD, 512], F32, tag="ssb")
                                nc.vector.tensor_copy(ssb[:], py[HD:P, :])
                                rec = ph3s.tile([HD, 512], F32, tag="rec")
                                nc.vector.reciprocal_approx_fast(rec[:], ssb[:])
                                yt = ph3s.tile([HD, 512], BF16, tag="yt")
                                nc.vector.tensor_tensor(
                                    yt[:], py[0:HD, :], rec[:], op=Alu.mult
                                )
                                t0 = b * T + tq * 512
                                nc.sync.dma_start(
                                    yT_send[h][t0 // TOWN, :,
                                               t0 % TOWN : t0 % TOWN + 512],
                                    yt[:],
                                )
                        with nc.named_scope(f"cc_a2a_y{h}"):
                            nc.gpsimd.collective_compute(
                                "AllToAll", Alu.bypass, replica_groups=groups,
                                ins=[yT_send[h][:]], outs=[yT_recv[h][:]],
                            )

                # =====================================================
                # Phase 4: proj (bf16) + residual + LN2 -> h2^T bf16
                # (attention SBUF tiles freed first)
                # =====================================================
                attn_scope.__exit__(None, None, None)
                with tc.tile_pool(name="keep", bufs=1) as keep:
                    h2T = keep.tile([P, 8, 8, P], BF16)  # [p, dblk, t, j]
                    x1_sb = keep.tile([P, 8, D], F32)    # [p, t, d]
                    with (
                        tc.tile_pool(name="ph4", bufs=3) as ph4,
                        tc.tile_pool(name="xtp", bufs=1) as xtp,
                        tc.tile_pool(name="psC", bufs=4, space="PSUM") as psC,
                        tc.tile_pool(name="psD", bufs=3, space="PSUM") as psD,
                        nc.named_scope("ph4_proj_ln2"),
                    ):
                        # yT_recv[h][ko, p, t]: y-dim = 128*ko + 64*h + p
                        yfv0 = yT_recv[0].rearrange("ko p t -> p ko t")
                        yfv1 = yT_recv[1].rearrange("ko p t -> p ko t")
                        # x_res loads don't depend on the y AllToAll: issue
                        # them all up front so they run during the y1 wait
                        xts = []
                        for t in range(8):
                            xt = xtp.tile([P, D], F32, tag=f"xt4_{t}",
                                          name=f"xt4_{t}")
                            nc.sync.dma_start(xt[:], x_res[t * P : (t + 1) * P, :])
                            xts.append(xt)
                        # b_fc2 broadcast [P, D] (real scale), built once
                        # here where PE idles on the y1 AllToAll; folded
                        # into x1 below so ph5's fc2 PSUM chains start
                        # directly on the DR matmuls (16 fewer matmuls in
                        # the 94%-PE-busy ph5 window)
                        bfc2bc = keep.tile([P, D], F32)
                        for n in range(2):
                            pb = psC.tile([P, 512], F32, tag="pj")
                            nc.tensor.matmul(
                                pb[:], ones_b[:, :P],
                                bfc2_sb[:, n * 512 : (n + 1) * 512],
                                start=True, stop=True,
                            )
                            nc.vector.tensor_scalar(
                                bfc2bc[:, n * 512 : (n + 1) * 512], pb[:],
                                1.0 / SF2, None, Alu.mult,
                            )
                        for t in range(8):
                            yt_own = ph4.tile([P, 8, P], BF16, tag="ytown")
                            nc.sync.dma_start(
                                yt_own[0:HD, :, :], yfv0[:, :, t * P : (t + 1) * P]
                            )
                            nc.sync.dma_start(
                                yt_own[HD:P, :, :], yfv1[:, :, t * P : (t + 1) * P]
                            )
                            xt = xts[t]
                            for n in range(2):
                                pp = psC.tile([P, 512], F32, tag="pj")
                                for ko in range(8):
                                    nc.tensor.matmul(
                                        pp[:], yt_own[:, ko, :],
                                        wproj_sb[:, ko, n * 512 : (n + 1) * 512],
                                        start=(ko == 0), stop=(ko == 7),
                                    )
                                nc.vector.tensor_tensor(
                                    x1_sb[:, t, n * 512 : (n + 1) * 512], pp[:],
                                    xt[:, n * 512 : (n + 1) * 512], op=Alu.add,
                                )
                            layernorm_std(ph4, x1_sb[:, t, :], psD, h2T, t)
                            nc.vector.tensor_tensor(
                                x1_sb[:, t, :], x1_sb[:, t, :], bfc2bc[:],
                                op=Alu.add,
                            )

                    # =====================================================
                    # Phase 5: MLP — fc bf16 (streamed weights), gelu -> m
                    # fp8, fc2 via fp8 DoubleRow (2 token groups of 512)
                    # =====================================================
                    with (
                        tc.tile_pool(name="mt", bufs=2) as mtp,
                        tc.tile_pool(name="ph5", bufs=3) as ph5,
                        tc.tile_pool(name="psM", bufs=2, space="PSUM") as psM,
                        tc.tile_pool(name="psO", bufs=1, space="PSUM") as psO,
                        nc.named_scope("ph5_mlp"),
                    ):
                        wfcv = wfc.rearrange("(ko p) n -> p ko n", p=P)
                        h2flat = h2T[:].rearrange("p dblk t j -> p dblk (t j)")
                        for g in range(2):
                            mT = mtp.tile([P, DFF // P, 512], FP8, tag="mt")
                            for kb in range(DFF // P):
                                wt = ph5.tile([P, 8, P], BF16, tag="wfct")
                                nc.sync.dma_start(
                                    wt[:], wfcv[:, :, kb * P : (kb + 1) * P]
                                )
                                pm = psM.tile([P, 512], F32, tag="pm")
                                for ko in range(8):
                                    nc.tensor.matmul(
                                        pm[:], wt[:, ko, :],
                                        h2flat[:, ko, g * 512 : (g + 1) * 512],
                                        start=(ko == 0), stop=(ko == 7),
                                    )
                                nc.scalar.activation(
                                    mT[:, kb, :], pm[:], Act.Gelu,
                                    bias=bfc_sb[:, kb : kb + 1],
                                )
                            mpair = mT[:].rearrange("p (u i) t -> p u i t", i=2)
                            for n2 in range(2):
                                pos = [
                                    psO.tile([P, 512], F32, tag=f"po{t2}",
                                             name=f"po_{g}_{n2}_{t2}")
                                    for t2 in range(4)
                                ]
                                for u in range(DFF // 256):
                                    for t2 in range(4):
                                        nc.tensor.matmul(
                                            pos[t2][:],
                                            mpair[:, u, :, t2 * P : (t2 + 1) * P],
                                            wfc2_sb[:, u, :, n2 * 512 : (n2 + 1) * 512],
                                            start=(u == 0), stop=(u == DFF // 256 - 1),
                                            perf_mode=DR,
                                            skip_group_check=True,
                                        )
                                for t2 in range(4):
                                    t = 4 * g + t2
                                    ot = ph5.tile([P, 512], F32, tag="ot")
                                    nc.vector.scalar_tensor_tensor(
                                        ot[:], pos[t2][:], 1.0 / SF2,
                                        x1_sb[:, t, n2 * 512 : (n2 + 1) * 512],
                                        op0=Alu.mult, op1=Alu.add,
                                    )
                                    nc.sync.dma_start(
                                        out_own[t * P : (t + 1) * P,
                                                n2 * 512 : (n2 + 1) * 512],
                                        ot[:],
                                    )

    nc.compile()
    return nc


_NC_CACHE = None
_last_in_maps = None


def _pow2scale(w, target=192.0):
    m = float(np.abs(w).max())
    if m <= 0:
        return 1.0
    return float(2.0 ** np.floor(np.log2(target / m)))


def _pack_pair(w, groups):
    """[K, M] -> [128, K//256, 2, M] with k = kp*256 + i*128 + p."""
    K, M = w.shape
    assert K == groups * 256
    return np.ascontiguousarray(
        w.reshape(groups, 2, P, M).transpose(2, 0, 1, 3).astype(F8))


def kernel(x, ln1_w, ln1_b, ln2_w, ln2_b, w_attn, b_attn, w_proj, b_proj,
           w_fc, b_fc, w_fc2, b_fc2):
    global _NC_CACHE, _last_in_maps

    f = np.ascontiguousarray
    x = np.asarray(x, np.float32)
    xf = x.reshape(TOK, D)
    ln1_w = np.asarray(ln1_w, np.float32)
    ln1_b = np.asarray(ln1_b, np.float32)
    ln2_w = np.asarray(ln2_w, np.float32)
    ln2_b = np.asarray(ln2_b, np.float32)
    w_attn = np.asarray(w_attn, np.float32)
    b_attn = np.asarray(b_attn, np.float32)
    w_proj = np.asarray(w_proj, np.float32)
    b_proj = np.asarray(b_proj, np.float32)
    w_fc = np.asarray(w_fc, np.float32)
    b_fc = np.asarray(b_fc, np.float32)
    w_fc2 = np.asarray(w_fc2, np.float32)
    b_fc2 = np.asarray(b_fc2, np.float32)

    # fold LN affine params into the following matmuls
    w_attn_eff = ln1_w[:, None] * w_attn
    b_attn_eff = b_attn + ln1_b @ w_attn
    w_fc_eff = ln2_w[:, None] * w_fc
    b_fc_eff = b_fc + ln2_b @ w_fc

    SF2 = _pow2scale(w_fc2)
    SFQ = _pow2scale(w_attn_eff)

    if _NC_CACHE is None:
        _NC_CACHE = build(SF2, SFQ)
    nc = _NC_CACHE

    def bf(v):
        return f(np.asarray(v, np.float32).astype(BF))

    wfc28 = _pack_pair(w_fc2 * SF2, 16)
    bfc2s = f((SF2 * b_fc2).reshape(1, D).astype(BF))
    bfc_strip = f(b_fc_eff.reshape(DFF // P, P).T)

    in_maps = []
    for c in range(NCORES):
        cols = slice(P * c, P * (c + 1))
        bq_c = b_attn_eff[cols.start : cols.stop]
        bk_c = b_attn_eff[D + cols.start : D + cols.stop]
        bv_c = b_attn_eff[2 * D + cols.start : 2 * D + cols.stop]
        qsc_c = np.zeros((P, 2), np.float32)
        qbi_c = np.zeros((P, 2), np.float32)
        for h in range(2):
            rows = slice(HD * h, HD * (h + 1))
            qsc_c[rows, h] = 1.0 / SFQ
            qbi_c[rows, h] = bq_c[rows]
        xo = xf[c * TOWN : (c + 1) * TOWN]
        in_maps.append({
            "x_own": f(xo),
            "x_res": f(xo + b_proj[None, :]),
            "wq": _pack_pair(SFQ * w_attn_eff[:, cols], 4),
            "wk": _pack_pair(SFQ * w_attn_eff[:, D + cols.start : D + cols.stop], 4),
            "wv": _pack_pair(SFQ * w_attn_eff[:, 2 * D + cols.start : 2 * D + cols.stop], 4),
            "qsc": qsc_c,
            "qbi": qbi_c,
            "bk": f(bk_c.reshape(P, 1)),
            "bv": f(bv_c.reshape(P, 1)),
            "wproj": bf(w_proj),
            "wfc": bf(w_fc_eff),
            "bfc": bfc_strip,
            "wfc28": wfc28,
            "bfc2s": bfc2s,
        })

    _last_in_maps = in_maps
    res = run_bass_kernel_spmd(nc, in_maps, core_ids=list(range(NCORES)))
    out = np.concatenate([res.results[c]["out"] for c in range(NCORES)], axis=0)
    return out.reshape(B, T, D)

